# revision 4
# baseline (speedup 1.0000x reference)
"""Trainium2 Bass kernel for the AdditiveModel reduction.

Computes out[y] = sum_{q,p} c[y,q] * a[y,q,p] * dot(lam[y,q,p,:], x[q,p,:])
with Y=16, Q=8, P=32, D=8192 (lam is 128 MiB -> memory-bound).

Sharding: one q per core (Q == 8 cores). Each core is fully independent and
produces a partial out[16]; the host sums the 8 partials at gather time.

Per-core compute: the D-axis dot products run on the TensorEngine. At
sharding time the host hands each core its lam slice pre-transposed to
[d, (y,p)] (d on partitions), so the dots become 64 PSUM-accumulated
matmuls lhsT=x[dchunk, p] (128x32), rhs=lam[dchunk, (y,p)] (128x512).
PSUM then holds G[m, (y,p)] = dot(x[p_m,:], lam[y,p,:]); a masked
diagonal extraction + (c*a) weighting + ones-matmul collapse yields out.
"""

from contextlib import ExitStack

import numpy as np

Y, Q, P, D = 16, 8, 32, 8192
NCORES = 8
KC = 128                 # contraction chunk (partition count)
DC = D // KC             # 64 d-chunks
SLABS = 8                # lam streamed in 8 slabs of 2 MiB
CPS = DC // SLABS        # chunks per slab
YP = Y * P               # 512

_CACHE = {}


def _build_nc():
    import concourse.bass as bass
    import concourse.mybir as mybir
    import concourse.tile as tile
    from concourse import bacc

    f32 = mybir.dt.float32
    nc = bacc.Bacc(None, target_bir_lowering=False)

    lamT = nc.declare_dram_parameter("lamT", [KC, DC * YP], f32, isOutput=False)
    xT = nc.declare_dram_parameter("xT", [KC, DC * P], f32, isOutput=False)
    aT = nc.declare_dram_parameter("aT", [P, Y], f32, isOutput=False)
    crep = nc.declare_dram_parameter("crep", [P, Y], f32, isOutput=False)
    m0 = nc.declare_dram_parameter("m0", [P, YP], f32, isOutput=False)
    ones = nc.declare_dram_parameter("ones", [P, 1], f32, isOutput=False)
    out = nc.declare_dram_parameter("out", [1, Y], f32, isOutput=True)

    with tile.TileContext(nc) as tc, ExitStack() as ctx:
        const = ctx.enter_context(tc.tile_pool(name="const", bufs=1))
        slab_pool = ctx.enter_context(tc.tile_pool(name="slab", bufs=2))
        psum_pool = ctx.enter_context(
            tc.tile_pool(name="psum", bufs=1, space=bass.MemorySpace.PSUM)
        )
        tailp = ctx.enter_context(tc.tile_pool(name="tail", bufs=1))

        x_sb = const.tile([KC, DC * P], f32)
        nc.sync.dma_start(x_sb[:], xT[:])
        m0_sb = const.tile([P, YP], f32)
        nc.sync.dma_start(m0_sb[:], m0[:])
        aT_sb = const.tile([P, Y], f32)
        nc.sync.dma_start(aT_sb[:], aT[:])
        cr_sb = const.tile([P, Y], f32)
        nc.sync.dma_start(cr_sb[:], crep[:])
        on_sb = const.tile([P, 1], f32)
        nc.sync.dma_start(on_sb[:], ones[:])

        wT = const.tile([P, Y], f32)
        nc.vector.tensor_mul(wT[:], aT_sb[:], cr_sb[:])

        proj = psum_pool.tile([P, YP], f32)
        for s in range(SLABS):
            slab = slab_pool.tile([KC, CPS * YP], f32)
            nc.sync.dma_start(slab[:], lamT[:, s * CPS * YP:(s + 1) * CPS * YP])
            for c in range(CPS):
                cg = s * CPS + c
                nc.tensor.matmul(
                    proj[:],
                    x_sb[:, cg * P:(cg + 1) * P],
                    slab[:, c * YP:(c + 1) * YP],
                    start=(cg == 0),
                    stop=(cg == DC - 1),
                )

        # diag mask: keep only m == p entries of G[m, (y,p)]
        t2 = tailp.tile([P, YP], f32)
        nc.vector.tensor_mul(t2[:], proj[:], m0_sb[:])
        # sum each 32-wide p-group -> S[m, y] = proj[y, m]
        s_t = tailp.tile([P, Y], f32)
        nc.vector.reduce_sum(
            s_t[:],
            t2[:].rearrange("m (y p) -> m y p", p=P),
            axis=mybir.AxisListType.X,
        )
        # weight by c*a and collapse partitions with a ones-matvec
        sw = tailp.tile([P, Y], f32)
        nc.vector.tensor_mul(sw[:], s_t[:], wT[:])
        outp = psum_pool.tile([1, Y], f32)
        nc.tensor.matmul(outp[:], on_sb[:], sw[:], start=True, stop=True)
        out_sb = tailp.tile([1, Y], f32)
        nc.vector.tensor_copy(out_sb[:], outp[:])
        nc.sync.dma_start(out[:], out_sb[:])

    nc.compile()
    return nc


def _shard_inputs(x, lam, a, c):
    """Per-core input maps. Pure slicing/layout transforms only."""
    m0_np = np.tile(np.eye(P, dtype=np.float32), (1, Y))          # [P, Y*P]
    ones_np = np.ones((P, 1), dtype=np.float32)
    in_maps = []
    for q in range(NCORES):
        lam_q = lam[:, q]                                          # [Y, P, D]
        lamT = np.ascontiguousarray(
            lam_q.transpose(2, 0, 1).reshape(DC, KC, YP)
            .transpose(1, 0, 2).reshape(KC, DC * YP)
        )
        x_q = x[q]                                                 # [P, D]
        xTn = np.ascontiguousarray(
            x_q.T.reshape(DC, KC, P).transpose(1, 0, 2).reshape(KC, DC * P)
        )
        aTn = np.ascontiguousarray(a[:, q].T)                      # [P, Y]
        crn = np.ascontiguousarray(
            np.broadcast_to(c[:, q][None, :], (P, Y))
        ).astype(np.float32)
        in_maps.append(
            {
                "lamT": lamT.astype(np.float32, copy=False),
                "xT": xTn.astype(np.float32, copy=False),
                "aT": aTn.astype(np.float32, copy=False),
                "crep": crn,
                "m0": m0_np,
                "ones": ones_np,
            }
        )
    return in_maps


def get_nc():
    if "nc" not in _CACHE:
        _CACHE["nc"] = _build_nc()
    return _CACHE["nc"]


def run(x, lam, a, c, trace=False, **spmd_kwargs):
    from concourse.bass_utils import run_bass_kernel_spmd

    nc = get_nc()
    in_maps = _shard_inputs(
        np.asarray(x), np.asarray(lam), np.asarray(a), np.asarray(c)
    )
    res = run_bass_kernel_spmd(
        nc, in_maps, core_ids=list(range(NCORES)), trace=trace, **spmd_kwargs
    )
    out = np.zeros((Y,), dtype=np.float32)
    for core_res in res.results:
        out += core_res["out"].reshape(Y)
    return out, res


def kernel(x, lam, a, c):
    out, _ = run(x, lam, a, c, trace=False)
    return out


# revision 5
# speedup vs baseline: 1.4747x; 1.4747x over previous
"""Trainium2 Bass kernel for the AdditiveModel reduction.

Computes out[y] = sum_{q,p} c[y,q] * a[y,q,p] * dot(lam[y,q,p,:], x[q,p,:])
with Y=16, Q=8, P=32, D=8192 (lam is 128 MiB -> memory-bound).

Sharding: one q per core (Q == 8 cores). Each core is fully independent and
produces a partial out[16]; the host sums the 8 partials at gather time.

Per-core compute: the D-axis dot products run on the TensorEngine. At
sharding time the host hands each core its lam slice pre-transposed to
[d, (y,p)] (d on partitions), so the dots become 64 PSUM-accumulated
matmuls lhsT=x[dchunk, p] (128x32), rhs=lam[dchunk, (y,p)] (128x512).
PSUM then holds G[m, (y,p)] = dot(x[p_m,:], lam[y,p,:]); a masked
diagonal extraction + (c*a) weighting + ones-matmul collapse yields out.
"""

from contextlib import ExitStack

import numpy as np

Y, Q, P, D = 16, 8, 32, 8192
NCORES = 8
KC = 128                 # contraction chunk (partition count)
DC = D // KC             # 64 d-chunks
SLABS = 8                # lam streamed in 8 slabs of 2 MiB
CPS = DC // SLABS        # chunks per slab
YP = Y * P               # 512

_CACHE = {}


def _build_nc():
    import concourse.bass as bass
    import concourse.mybir as mybir
    import concourse.tile as tile
    from concourse import bacc

    f32 = mybir.dt.float32
    nc = bacc.Bacc(None, target_bir_lowering=False)

    f32r = mybir.dt.float32r
    lamT = nc.declare_dram_parameter("lamT", [KC, DC * YP], f32r, isOutput=False)
    xT = nc.declare_dram_parameter("xT", [KC, DC * P], f32r, isOutput=False)
    aT = nc.declare_dram_parameter("aT", [P, Y], f32, isOutput=False)
    crep = nc.declare_dram_parameter("crep", [P, Y], f32, isOutput=False)
    m0 = nc.declare_dram_parameter("m0", [P, YP], f32, isOutput=False)
    ones = nc.declare_dram_parameter("ones", [P, 1], f32, isOutput=False)
    out = nc.declare_dram_parameter("out", [1, Y], f32, isOutput=True)

    with tile.TileContext(nc) as tc, ExitStack() as ctx:
        const = ctx.enter_context(tc.tile_pool(name="const", bufs=1))
        slab_pool = ctx.enter_context(tc.tile_pool(name="slab", bufs=2))
        psum_pool = ctx.enter_context(
            tc.tile_pool(name="psum", bufs=1, space=bass.MemorySpace.PSUM)
        )
        tailp = ctx.enter_context(tc.tile_pool(name="tail", bufs=1))

        x_sb = const.tile([KC, DC * P], f32r)
        nc.sync.dma_start(x_sb[:], xT[:])
        m0_sb = const.tile([P, YP], f32)
        nc.sync.dma_start(m0_sb[:], m0[:])
        aT_sb = const.tile([P, Y], f32)
        nc.sync.dma_start(aT_sb[:], aT[:])
        cr_sb = const.tile([P, Y], f32)
        nc.sync.dma_start(cr_sb[:], crep[:])
        on_sb = const.tile([P, 1], f32)
        nc.sync.dma_start(on_sb[:], ones[:])

        wT = const.tile([P, Y], f32)
        nc.vector.tensor_mul(wT[:], aT_sb[:], cr_sb[:])

        proj = psum_pool.tile([P, YP], f32)
        for s in range(SLABS):
            slab = slab_pool.tile([KC, CPS * YP], f32r)
            nc.sync.dma_start(slab[:], lamT[:, s * CPS * YP:(s + 1) * CPS * YP])
            for c in range(CPS):
                cg = s * CPS + c
                nc.tensor.matmul(
                    proj[:],
                    x_sb[:, cg * P:(cg + 1) * P],
                    slab[:, c * YP:(c + 1) * YP],
                    start=(cg == 0),
                    stop=(cg == DC - 1),
                )

        # diag mask: keep only m == p entries of G[m, (y,p)]
        t2 = tailp.tile([P, YP], f32)
        nc.vector.tensor_mul(t2[:], proj[:], m0_sb[:])
        # sum each 32-wide p-group -> S[m, y] = proj[y, m]
        s_t = tailp.tile([P, Y], f32)
        nc.vector.reduce_sum(
            s_t[:],
            t2[:].rearrange("m (y p) -> m y p", p=P),
            axis=mybir.AxisListType.X,
        )
        # weight by c*a and collapse partitions with a ones-matvec
        sw = tailp.tile([P, Y], f32)
        nc.vector.tensor_mul(sw[:], s_t[:], wT[:])
        outp = psum_pool.tile([1, Y], f32)
        nc.tensor.matmul(outp[:], on_sb[:], sw[:], start=True, stop=True)
        out_sb = tailp.tile([1, Y], f32)
        nc.vector.tensor_copy(out_sb[:], outp[:])
        nc.sync.dma_start(out[:], out_sb[:])

    nc.compile()
    return nc


def _shard_inputs(x, lam, a, c):
    """Per-core input maps. Pure slicing/layout transforms only."""
    m0_np = np.tile(np.eye(P, dtype=np.float32), (1, Y))          # [P, Y*P]
    ones_np = np.ones((P, 1), dtype=np.float32)
    in_maps = []
    for q in range(NCORES):
        lam_q = lam[:, q]                                          # [Y, P, D]
        lamT = np.ascontiguousarray(
            lam_q.transpose(2, 0, 1).reshape(DC, KC, YP)
            .transpose(1, 0, 2).reshape(KC, DC * YP)
        )
        x_q = x[q]                                                 # [P, D]
        xTn = np.ascontiguousarray(
            x_q.T.reshape(DC, KC, P).transpose(1, 0, 2).reshape(KC, DC * P)
        )
        aTn = np.ascontiguousarray(a[:, q].T)                      # [P, Y]
        crn = np.ascontiguousarray(
            np.broadcast_to(c[:, q][None, :], (P, Y))
        ).astype(np.float32)
        in_maps.append(
            {
                "lamT": lamT.astype(np.float32, copy=False),
                "xT": xTn.astype(np.float32, copy=False),
                "aT": aTn.astype(np.float32, copy=False),
                "crep": crn,
                "m0": m0_np,
                "ones": ones_np,
            }
        )
    return in_maps


def get_nc():
    if "nc" not in _CACHE:
        _CACHE["nc"] = _build_nc()
    return _CACHE["nc"]


def run(x, lam, a, c, trace=False, **spmd_kwargs):
    from concourse.bass_utils import run_bass_kernel_spmd

    nc = get_nc()
    in_maps = _shard_inputs(
        np.asarray(x), np.asarray(lam), np.asarray(a), np.asarray(c)
    )
    res = run_bass_kernel_spmd(
        nc, in_maps, core_ids=list(range(NCORES)), trace=trace, **spmd_kwargs
    )
    out = np.zeros((Y,), dtype=np.float32)
    for core_res in res.results:
        out += core_res["out"].reshape(Y)
    return out, res


def kernel(x, lam, a, c):
    out, _ = run(x, lam, a, c, trace=False)
    return out
